# revision 4
# baseline (speedup 1.0000x reference)
"""CRTN middle_l query construction as a pure-DMA Bass kernel on 8 TRN2 cores.

Math (from the reference):
    query_base = concat([neighbor_mem[-1], wise_inputs], axis=0)   # (256, B, H)
    query[i, j] = query_base[i + j + 1]                            # (S, S, B, H)

For a fixed output row i, query[i] = query_base[i+1 : i+129] is one contiguous
8 MB slab of query_base.  The whole problem is a memory-bound replication:
16 MB of source fanned out to 1 GiB of output.

Sharding: over the output axis i (S=128 -> 16 rows per core).  Core k needs
query_base rows [16k+1, 16k+144) — 143 rows of 64 KB = 9.4 MB, staged once in
SBUF (row r -> partition r % 128, column r // 128).  Each output row i=16k+m
is then written with at most two SBUF->DRAM DMAs (the partition wrap at 128
splits the 128-row window into two rectangles).  Per-core HBM traffic:
9.4 MB read + 128 MB write ~= 137 MB -> ~385 us at ~358 GB/s/core.
"""

import numpy as np

import concourse.bacc as bacc
import concourse.bass as bass
import concourse.mybir as mybir
import concourse.tile as tile
from concourse.bass_utils import run_bass_kernel_spmd

# Problem shape (hardcoded; harness contract forbids reading spec.json here).
NEI_LEN = 128
S = 128
B = 16
H = 1024
N_CORES = 8
ROWS_PER_CORE = S // N_CORES          # 16 output rows (values of i) per core
IN_ROWS = ROWS_PER_CORE + S - 1       # 143 query_base rows needed per core
ROW_ELEMS = B * H                     # 16384 f32 = 64 KB per query_base row

# Profiling knobs for test harnesses: set TRACE=True before calling kernel()
# to capture an NTFF profile; LAST_EXEC_NS then holds the slowest-core HW time.
TRACE = False
LAST_EXEC_NS = None

_nc_cache = None


def _build_nc() -> bass.Bass:
    # Bacc (not raw Bass): its compile() pass splits multi-sem waits into
    # event-semaphore chains — the walrus codegen rejects instructions with
    # more than one sync wait ("Too many sync wait commands").
    nc = bacc.Bacc("TRN2", target_bir_lowering=False, debug=False)
    qb = nc.dram_tensor(
        "qb", [IN_ROWS, ROW_ELEMS], mybir.dt.float32, kind="ExternalInput"
    )
    out = nc.dram_tensor(
        "out", [ROWS_PER_CORE, S, ROW_ELEMS], mybir.dt.float32, kind="ExternalOutput"
    )
    with tile.TileContext(nc) as tc:
        with tc.tile_pool(name="stage", bufs=1) as pool:
            # Row r of qb lives at partition r % 128, column block r // 128.
            buf = pool.tile([128, 2 * ROW_ELEMS], mybir.dt.float32)
            nc.sync.dma_start(out=buf[:, 0:ROW_ELEMS], in_=qb[0:128, :])
            nc.sync.dma_start(
                out=buf[0 : IN_ROWS - 128, ROW_ELEMS : 2 * ROW_ELEMS],
                in_=qb[128:IN_ROWS, :],
            )
            for m in range(ROWS_PER_CORE):
                # Output row i=16k+m reads local rows m..m+127: partitions
                # m..127 of column 0, then (wrap) partitions 0..m-1 of col 1.
                nc.sync.dma_start(
                    out=out[m, 0 : 128 - m, :], in_=buf[m:128, 0:ROW_ELEMS]
                )
                if m > 0:
                    nc.sync.dma_start(
                        out=out[m, 128 - m : 128, :],
                        in_=buf[0:m, ROW_ELEMS : ROW_ELEMS + ROW_ELEMS],
                    )
    nc.compile()
    return nc


def kernel(neighbor_mem: np.ndarray, wise_inputs: np.ndarray) -> np.ndarray:
    global _nc_cache, LAST_EXEC_NS
    assert neighbor_mem.shape == (13, NEI_LEN, B, H), neighbor_mem.shape
    assert wise_inputs.shape == (S, B, H), wise_inputs.shape

    qb_full = np.concatenate(
        [
            np.asarray(neighbor_mem[-1], dtype=np.float32).reshape(NEI_LEN, ROW_ELEMS),
            np.asarray(wise_inputs, dtype=np.float32).reshape(S, ROW_ELEMS),
        ],
        axis=0,
    )  # (256, 16384)

    in_maps = [
        {"qb": qb_full[ROWS_PER_CORE * k + 1 : ROWS_PER_CORE * k + 1 + IN_ROWS]}
        for k in range(N_CORES)
    ]

    if _nc_cache is None:
        _nc_cache = _build_nc()

    res = run_bass_kernel_spmd(
        _nc_cache, in_maps, core_ids=list(range(N_CORES)), trace=TRACE
    )
    LAST_EXEC_NS = res.exec_time_ns

    out = np.concatenate(
        [r["out"].reshape(ROWS_PER_CORE, S, B, H) for r in res.results], axis=0
    )
    return out


# revision 5
# speedup vs baseline: 1.0166x; 1.0166x over previous
"""CRTN middle_l query construction as a pure-DMA Bass kernel on 8 TRN2 cores.

Math (from the reference):
    query_base = concat([neighbor_mem[-1], wise_inputs], axis=0)   # (256, B, H)
    query[i, j] = query_base[i + j + 1]                            # (S, S, B, H)

Each output "row" (i, j) is a 64 KB copy of one query_base row — the whole
problem is a memory-bound replication: 16 MB of source fanned out to 1 GiB of
output, so per-core time is bounded by HBM write bandwidth (~360 GB/s/core).

Sharding: a 4x2 block grid over (i, j) — core (a, b) handles i in
[32a, 32a+32), j in [64b, 64b+64).  That block only touches query_base rows
i+j+1 in [32a+64b+1, 32a+64b+96): 95 rows = 6.2 MB, staged once in SBUF (row
l -> partition l).  Each output row i' is then ONE contiguous SBUF->DRAM DMA
reading partitions i'..i'+63 (no partition wrap).  Per-core HBM traffic:
6.2 MB read + 128 MiB write -> ~390 us at ~360 GB/s/core.
(The 16-rows-per-core i-sharding variant stages 143 rows = 9.4 MB instead;
the block grid trades that down to 95 and keeps every core's program
identical, as SPMD requires.)
"""

import numpy as np

import concourse.bacc as bacc
import concourse.bass as bass
import concourse.mybir as mybir
import concourse.tile as tile
from concourse.bass_utils import run_bass_kernel_spmd

# Problem shape (hardcoded; harness contract forbids reading spec.json here).
NEI_LEN = 128
S = 128
B = 16
H = 1024
N_CORES = 8
GRID_A = 4                 # blocks along i
GRID_B = 2                 # blocks along j
BLK_I = S // GRID_A        # 32 output i-values per core
BLK_J = S // GRID_B        # 64 output j-values per core
IN_ROWS = BLK_I + BLK_J - 1  # 95 query_base rows needed per core
ROW_ELEMS = B * H          # 16384 f32 = 64 KB per query_base row

# Timing side-channel for test harnesses (exec_time_ns when a profile ran).
LAST_EXEC_NS = None

_nc_cache = None


def _build_nc() -> bass.Bass:
    # Bacc (not raw Bass): its compile() pass splits multi-sem waits into
    # event-semaphore chains — the walrus codegen rejects instructions with
    # more than one sync wait ("Too many sync wait commands").
    nc = bacc.Bacc("TRN2", target_bir_lowering=False, debug=False)
    qb = nc.dram_tensor(
        "qb", [IN_ROWS, ROW_ELEMS], mybir.dt.float32, kind="ExternalInput"
    )
    out = nc.dram_tensor(
        "out", [BLK_I, BLK_J, ROW_ELEMS], mybir.dt.float32, kind="ExternalOutput"
    )
    with tile.TileContext(nc) as tc:
        with tc.tile_pool(name="stage", bufs=1) as pool:
            # Row l of qb lives at partition l; all 95 rows fit in one column.
            buf = pool.tile([128, ROW_ELEMS], mybir.dt.float32)
            nc.sync.dma_start(out=buf[0:IN_ROWS, :], in_=qb[:, :])
            for i in range(BLK_I):
                # Output row i' reads local rows i'..i'+63 = partitions
                # i'..i'+63 — one rectangular AP, no wrap.
                nc.sync.dma_start(out=out[i, :, :], in_=buf[i : i + BLK_J, :])
    nc.compile()
    return nc


def kernel(neighbor_mem: np.ndarray, wise_inputs: np.ndarray) -> np.ndarray:
    global _nc_cache, LAST_EXEC_NS
    assert neighbor_mem.shape == (13, NEI_LEN, B, H), neighbor_mem.shape
    assert wise_inputs.shape == (S, B, H), wise_inputs.shape

    qb_full = np.concatenate(
        [
            np.asarray(neighbor_mem[-1], dtype=np.float32).reshape(NEI_LEN, ROW_ELEMS),
            np.asarray(wise_inputs, dtype=np.float32).reshape(S, ROW_ELEMS),
        ],
        axis=0,
    )  # (256, 16384)

    # Core k = (a, b) with a = k // GRID_B, b = k % GRID_B.
    in_maps = []
    for k in range(N_CORES):
        a, b = divmod(k, GRID_B)
        base = BLK_I * a + BLK_J * b + 1
        in_maps.append({"qb": qb_full[base : base + IN_ROWS]})

    if _nc_cache is None:
        _nc_cache = _build_nc()

    res = run_bass_kernel_spmd(_nc_cache, in_maps, core_ids=list(range(N_CORES)))
    LAST_EXEC_NS = res.exec_time_ns

    out = np.empty((S, S, B, H), dtype=np.float32)
    for k in range(N_CORES):
        a, b = divmod(k, GRID_B)
        out[BLK_I * a : BLK_I * (a + 1), BLK_J * b : BLK_J * (b + 1)] = res.results[
            k
        ]["out"].reshape(BLK_I, BLK_J, B, H)
    return out
